# revision 2
# baseline (speedup 1.0000x reference)
"""Trainium2 Bass kernel for nn_DiscreteConditionalEntropyModel (VQ codebook).

Computes, for params (4,16384,256), param_table (2048,256), logits (2048,):
  p   = clip(params, -1, 1)
  idx = argmin_d ||p - W_d||^2  == argmax_d (p.W_d - 0.5|W_d|^2)
  params_quantized = W[idx]
  log_pmf  = log_softmax(W[idx], axis=-1)
  param_bit = sum(log_softmax(logits)[idx]) / -ln2

Sharding: data-parallel over the 65536 positions across 8 NeuronCores
(8192 positions/core); param_table derived tensors replicated.

Device per 128-position tile:
  - scores via bf16x3 matmuls (p=hi+lo, W=Whi+Wlo; hi.Whi + hi.Wlo + lo.Whi)
    which matches fp32 argmin on this data, at 3/8 the fp32 PE cost
  - the -0.5|W|^2 bias is folded into the PSUM accumulation via a K=2
    ones x [-h_hi; -h_lo] matmul
  - DVE max + max_index read PSUM directly -> idx
  - one indirect-DMA gather of [W_row | log_softmax(W)_row | llp_row] rows
    (per-codebook-row functions precomputed on host)
"""
import math

import numpy as np
import ml_dtypes

LN2 = math.log(2.0)

B, N, C, D = 4, 16384, 256, 2048
NC_COUNT = 8
POS = B * N                   # 65536 total positions
PER = POS // NC_COUNT         # 8192 positions per core
TILE = 128
TILES = PER // TILE           # 64 tiles per core
CAUG = 528                    # gather row: 256 W + 256 logpmf + 1 llp + pad

_BF = ml_dtypes.bfloat16

_CACHE = {}
LAST_RESULTS = None           # BassKernelResults of the most recent run


def _split_bf16(x32):
    """x32 (f32) -> (hi, lo) bf16 with hi + lo ~= x to ~2^-17 rel."""
    hi = x32.astype(_BF)
    lo = (x32 - hi.astype(np.float32)).astype(_BF)
    return hi, lo


def _build():
    import concourse.bacc as bacc
    import concourse.mybir as mybir
    from concourse.tile import TileContext
    from concourse.bass import IndirectOffsetOnAxis

    F32 = mybir.dt.float32
    BF16 = mybir.dt.bfloat16
    U32 = mybir.dt.uint32

    nc = bacc.Bacc("TRN2", debug=False, enable_asserts=False)

    pth = nc.dram_tensor("pth", [2, 128, PER], BF16, kind="ExternalInput")
    ptl = nc.dram_tensor("ptl", [2, 128, PER], BF16, kind="ExternalInput")
    wth = nc.dram_tensor("wth", [2, 128, D], BF16, kind="ExternalInput")
    wtl = nc.dram_tensor("wtl", [2, 128, D], BF16, kind="ExternalInput")
    hb = nc.dram_tensor("hb", [2, D], BF16, kind="ExternalInput")
    ones2 = nc.dram_tensor("ones2", [2, TILE], BF16, kind="ExternalInput")
    tab = nc.dram_tensor("tab", [D, CAUG], F32, kind="ExternalInput")

    pq_out = nc.dram_tensor("pq_out", [PER, C], F32, kind="ExternalOutput")
    lp_out = nc.dram_tensor("lp_out", [PER, C], F32, kind="ExternalOutput")
    bits_out = nc.dram_tensor("bits_out", [TILE, 1], F32, kind="ExternalOutput")

    with TileContext(nc) as tc:
        with (
            tc.tile_pool(name="const", bufs=1) as cpool,
            tc.tile_pool(name="pt", bufs=3) as ptpool,
            tc.tile_pool(name="work", bufs=3) as wpool,
            tc.tile_pool(name="psum", bufs=2, space="PSUM") as pspool,
        ):
            wth_sb = [cpool.tile([128, D], BF16, name=f"wth{k}") for k in range(2)]
            wtl_sb = [cpool.tile([128, D], BF16, name=f"wtl{k}") for k in range(2)]
            hb_sb = cpool.tile([2, D], BF16, name="hb_sb")
            ones_sb = cpool.tile([2, TILE], BF16, name="ones_sb")
            for k in range(2):
                nc.sync.dma_start(out=wth_sb[k][:], in_=wth.ap()[k])
                nc.sync.dma_start(out=wtl_sb[k][:], in_=wtl.ap()[k])
            nc.sync.dma_start(out=hb_sb[:], in_=hb.ap())
            nc.sync.dma_start(out=ones_sb[:], in_=ones2.ap())

            acc = cpool.tile([TILE, 1], F32, name="acc")
            nc.vector.memset(acc[:], 0.0)

            for j in range(TILES):
                ssl = slice(j * TILE, (j + 1) * TILE)
                pth_t = [ptpool.tile([128, TILE], BF16, name=f"pth_t{k}_{j}",
                                     tag=f"pth{k}") for k in range(2)]
                ptl_t = [ptpool.tile([128, TILE], BF16, name=f"ptl_t{k}_{j}",
                                     tag=f"ptl{k}") for k in range(2)]
                for k in range(2):
                    nc.sync.dma_start(out=pth_t[k][:], in_=pth.ap()[k][:, ssl])
                    nc.sync.dma_start(out=ptl_t[k][:], in_=ptl.ap()[k][:, ssl])

                ps = pspool.tile([TILE, D], F32, tag="ps")
                combos = [
                    (pth_t[0], [wth_sb[0], wtl_sb[0]]),
                    (pth_t[1], [wth_sb[1], wtl_sb[1]]),
                    (ptl_t[0], [wth_sb[0]]),
                    (ptl_t[1], [wth_sb[1]]),
                ]
                for n in range(4):
                    sl = slice(n * 512, (n + 1) * 512)
                    first = True
                    for stat, movs in combos:
                        for mov in movs:
                            nc.tensor.matmul(out=ps[:, sl], lhsT=stat[:],
                                             rhs=mov[:, sl], start=first, stop=False)
                            first = False
                    nc.tensor.matmul(out=ps[:, sl], lhsT=ones_sb[:],
                                     rhs=hb_sb[:, sl], start=False, stop=True)

                m8 = wpool.tile([TILE, 8], F32, tag="m8")
                idx8 = wpool.tile([TILE, 8], U32, tag="idx8")
                nc.vector.max(out=m8[:], in_=ps[:])
                nc.vector.max_index(out=idx8[:], in_max=m8[:], in_values=ps[:])

                q = wpool.tile([TILE, CAUG], F32, tag="q")
                nc.gpsimd.indirect_dma_start(
                    out=q[:], out_offset=None, in_=tab.ap(),
                    in_offset=IndirectOffsetOnAxis(ap=idx8[:, 0:1], axis=0))

                nc.sync.dma_start(out=pq_out.ap()[ssl, :], in_=q[:, 0:C])
                nc.sync.dma_start(out=lp_out.ap()[ssl, :], in_=q[:, C:2 * C])
                nc.vector.tensor_add(acc[:], acc[:], q[:, 2 * C:2 * C + 1])

            nc.sync.dma_start(out=bits_out.ap(), in_=acc[:])

    nc.compile()
    return nc


def _get_program():
    if "nc" not in _CACHE:
        _CACHE["nc"] = _build()
    return _CACHE["nc"]


def kernel(params, param_table, logits, _trace=False):
    global LAST_RESULTS
    from concourse.bass_utils import run_bass_kernel_spmd

    params = np.asarray(params, dtype=np.float32)
    W = np.asarray(param_table, dtype=np.float32)
    logits64 = np.asarray(logits, dtype=np.float64)

    # ---- host-side shard + preprocess ----
    p = np.clip(params.reshape(POS, C), -1.0, 1.0)
    pT = np.ascontiguousarray(p.T)                      # [C, POS]
    pT_hi, pT_lo = _split_bf16(pT)

    wT = np.ascontiguousarray(W.T).reshape(2, 128, D)   # [C, D] chunked
    wth, wtl = _split_bf16(wT)

    W64 = W.astype(np.float64)
    h = 0.5 * (W64 * W64).sum(1)                        # 0.5|W_d|^2
    hh, hl = _split_bf16((-h).astype(np.float32))
    hb = np.stack([hh, hl])
    ones2 = np.ones((2, TILE), dtype=_BF)

    # per-codebook-row precomputes: log_softmax rows + logit log-pmf
    wmax = W64.max(axis=1, keepdims=True)
    lse = np.log(np.exp(W64 - wmax).sum(axis=1, keepdims=True)) + wmax
    lp_tab = (W64 - lse)
    lmax = logits64.max()
    llp = logits64 - (np.log(np.exp(logits64 - lmax).sum()) + lmax)

    tab = np.zeros((D, CAUG), np.float32)
    tab[:, 0:C] = W
    tab[:, C:2 * C] = lp_tab.astype(np.float32)
    tab[:, 2 * C] = llp.astype(np.float32)

    in_maps = []
    for c in range(NC_COUNT):
        csl = slice(c * PER, (c + 1) * PER)
        in_maps.append({
            "pth": np.ascontiguousarray(pT_hi[:, csl]).reshape(2, 128, PER),
            "ptl": np.ascontiguousarray(pT_lo[:, csl]).reshape(2, 128, PER),
            "wth": wth, "wtl": wtl, "hb": hb, "ones2": ones2, "tab": tab,
        })

    nc = _get_program()
    res = run_bass_kernel_spmd(nc, in_maps, core_ids=list(range(NC_COUNT)),
                               trace=_trace)
    LAST_RESULTS = res

    # ---- gather / unshard ----
    pq = np.concatenate([r["pq_out"] for r in res.results], axis=0)
    lp = np.concatenate([r["lp_out"] for r in res.results], axis=0)
    bits = np.concatenate([r["bits_out"] for r in res.results], axis=0)

    log_pmf = lp.reshape(B, N, C)
    params_quantized = pq.reshape(B, N, C)
    param_bit = np.float32(bits.astype(np.float64).sum() / (-LN2))
    return log_pmf, params_quantized, param_bit


# revision 6
# speedup vs baseline: 268.3646x; 268.3646x over previous
"""Trainium2 Bass kernel for nn_DiscreteConditionalEntropyModel (VQ codebook).

Computes, for params (4,16384,256), param_table (2048,256), logits (2048,):
  p   = clip(params, -1, 1)
  idx = argmin_d ||p - W_d||^2  == argmax_d (p.W_d - 0.5|W_d|^2)
  params_quantized = W[idx]
  log_pmf  = log_softmax(W[idx], axis=-1)
  param_bit = sum(log_softmax(logits)[idx]) / -ln2

Sharding: data-parallel over the 65536 positions across 8 NeuronCores
(8192 positions/core); param_table derived tensors replicated.

Device per 128-position tile:
  - scores via bf16x3 matmuls (p=hi+lo, W=Whi+Wlo; hi.Whi + hi.Wlo + lo.Whi)
    which reproduces the fp32 argmin exactly on this data at 3/8 the
    fp32-matmul PE cost
  - the -0.5|W_d|^2 bias is folded into the PSUM accumulation via a K=2
    ones x [-h_hi; -h_lo] matmul, so no separate DVE bias pass is needed
  - DVE max + max_index read PSUM directly -> argmax index
  - one indirect-DMA gather of [W_row | log_softmax(W)_row | llp] rows
    (per-codebook-row functions precomputed on host), stored straight out
"""
import math

import numpy as np
import ml_dtypes

LN2 = math.log(2.0)

B, N, C, D = 4, 16384, 256, 2048
NC_COUNT = 8
POS = B * N                   # 65536 total positions
PER = POS // NC_COUNT         # 8192 positions per core
TILE = 128
TILES = PER // TILE           # 64 tiles per core
CAUG = 528                    # gather row: 256 W + 256 logpmf + 1 llp + pad

_BF = ml_dtypes.bfloat16

_CACHE = {}
LAST_RESULTS = None           # BassKernelResults of the most recent run


def _split_bf16(x32):
    """x32 (f32) -> (hi, lo) bf16 with hi + lo ~= x to ~2^-17 rel."""
    hi = x32.astype(_BF)
    lo = (x32 - hi.astype(np.float32)).astype(_BF)
    return hi, lo


def _build(repeat=1):
    """repeat>1 wraps the tile pipeline in a device-side loop re-processing
    the same data -- used only by the timing harness."""
    import concourse.bacc as bacc
    import concourse.mybir as mybir
    from concourse.tile import TileContext
    from concourse.bass import IndirectOffsetOnAxis

    F32 = mybir.dt.float32
    BF16 = mybir.dt.bfloat16
    U32 = mybir.dt.uint32

    nc = bacc.Bacc("TRN2", debug=False, enable_asserts=False)

    # pt: packed [c_within=128, chunk=4 (hi0,hi1,lo0,lo1), pos]
    pt = nc.dram_tensor("pt", [128, 4, PER], BF16, kind="ExternalInput")
    wth = nc.dram_tensor("wth", [2, 128, D], BF16, kind="ExternalInput")
    wtl = nc.dram_tensor("wtl", [2, 128, D], BF16, kind="ExternalInput")
    hb = nc.dram_tensor("hb", [2, D], BF16, kind="ExternalInput")
    ones2 = nc.dram_tensor("ones2", [2, TILE], BF16, kind="ExternalInput")
    tab = nc.dram_tensor("tab", [D, CAUG], F32, kind="ExternalInput")

    # packed output: [pos, 0:256]=params_quantized, [pos, 256:512]=log_pmf
    out2 = nc.dram_tensor("out2", [PER, 2 * C], F32, kind="ExternalOutput")
    bits_out = nc.dram_tensor("bits_out", [TILE, 1], F32, kind="ExternalOutput")

    with TileContext(nc) as tc:
        with (
            tc.tile_pool(name="const", bufs=1) as cpool,
            tc.tile_pool(name="pt", bufs=4) as ptpool,
            tc.tile_pool(name="work", bufs=4) as wpool,
            tc.tile_pool(name="psum", bufs=2, space="PSUM") as pspool,
        ):
            wth_sb = [cpool.tile([128, D], BF16, name=f"wth{k}") for k in range(2)]
            wtl_sb = [cpool.tile([128, D], BF16, name=f"wtl{k}") for k in range(2)]
            hb_sb = cpool.tile([2, D], BF16, name="hb_sb")
            ones_sb = cpool.tile([2, TILE], BF16, name="ones_sb")
            for k in range(2):
                nc.sync.dma_start(out=wth_sb[k][:], in_=wth.ap()[k])
                nc.sync.dma_start(out=wtl_sb[k][:], in_=wtl.ap()[k])
            nc.sync.dma_start(out=hb_sb[:], in_=hb.ap())
            nc.sync.dma_start(out=ones_sb[:], in_=ones2.ap())

            acc = cpool.tile([TILE, 1], F32, name="acc")
            nc.vector.memset(acc[:], 0.0)

            def _tile_loop():
                for j in range(TILES):
                    _tile_body(j)

            def _tile_body(j):
                ssl = slice(j * TILE, (j + 1) * TILE)
                # one DMA: all four 128-col chunks of this tile's p^T hi/lo
                pt_t = ptpool.tile([128, 4 * TILE], BF16, tag="pt", name=f"pt_{j}")
                nc.sync.dma_start(
                    out=pt_t[:], in_=pt.ap()[:, :, ssl])

                ps = pspool.tile([TILE, D], F32, tag="ps", name=f"ps_{j}")
                # stationary-outer order: 5 weight loads per tile
                stats = [
                    (pt_t[:, 0 * TILE:1 * TILE], [wth_sb[0], wtl_sb[0]]),
                    (pt_t[:, 1 * TILE:2 * TILE], [wth_sb[1], wtl_sb[1]]),
                    (pt_t[:, 2 * TILE:3 * TILE], [wth_sb[0]]),
                    (pt_t[:, 3 * TILE:4 * TILE], [wth_sb[1]]),
                ]
                n_stat = len(stats)
                for si, (stat, movs) in enumerate(stats):
                    for mov in movs:
                        for n in range(4):
                            sl = slice(n * 512, (n + 1) * 512)
                            nc.tensor.matmul(out=ps[:, sl], lhsT=stat,
                                             rhs=mov[:, sl],
                                             start=(si == 0 and mov is movs[0]),
                                             stop=False)
                for n in range(4):
                    sl = slice(n * 512, (n + 1) * 512)
                    nc.tensor.matmul(out=ps[:, sl], lhsT=ones_sb[:],
                                     rhs=hb_sb[:, sl], start=False, stop=True)

                m8 = wpool.tile([TILE, 8], F32, tag="m8", name=f"m8_{j}")
                idx8 = wpool.tile([TILE, 8], U32, tag="idx8", name=f"idx8_{j}")
                nc.vector.max(out=m8[:], in_=ps[:])
                nc.vector.max_index(out=idx8[:], in_max=m8[:], in_values=ps[:])

                q = wpool.tile([TILE, CAUG], F32, tag="q", name=f"q_{j}")
                nc.gpsimd.indirect_dma_start(
                    out=q[:], out_offset=None, in_=tab.ap(),
                    in_offset=IndirectOffsetOnAxis(ap=idx8[:, 0:1], axis=0))

                # pq and log_pmf are adjacent in the gather row: one store
                nc.sync.dma_start(out=out2.ap()[ssl, :], in_=q[:, 0:2 * C])
                nc.vector.tensor_add(acc[:], acc[:], q[:, 2 * C:2 * C + 1])

            if repeat == 1:
                _tile_loop()
            else:
                with tc.For_i(0, repeat, 1):
                    _tile_loop()

            nc.sync.dma_start(out=bits_out.ap(), in_=acc[:])

    nc.compile()
    return nc


def _get_program():
    if "nc" not in _CACHE:
        _CACHE["nc"] = _build()
    return _CACHE["nc"]


def _host_prep(params, param_table, logits):
    W = np.asarray(param_table, dtype=np.float32)
    logits64 = np.asarray(logits, dtype=np.float64)

    p = np.clip(np.asarray(params, dtype=np.float32).reshape(POS, C), -1.0, 1.0)
    pT = np.ascontiguousarray(p.T)                      # [C, POS]
    pT_hi, pT_lo = _split_bf16(pT)
    # packed [128, 4, POS]: chunks (hi c0-127, hi c128-255, lo c0-127, lo c128-255)
    pt_pack = np.empty((128, 4, POS), dtype=_BF)
    pt_pack[:, 0] = pT_hi[:128]
    pt_pack[:, 1] = pT_hi[128:]
    pt_pack[:, 2] = pT_lo[:128]
    pt_pack[:, 3] = pT_lo[128:]

    wT = np.ascontiguousarray(W.T).reshape(2, 128, D)
    wth, wtl = _split_bf16(wT)

    W64 = W.astype(np.float64)
    h = 0.5 * (W64 * W64).sum(1)
    hh, hl = _split_bf16((-h).astype(np.float32))
    hb = np.stack([hh, hl])
    ones2 = np.ones((2, TILE), dtype=_BF)

    wmax = W64.max(axis=1, keepdims=True)
    lse = np.log(np.exp(W64 - wmax).sum(axis=1, keepdims=True)) + wmax
    lp_tab = W64 - lse
    lmax = logits64.max()
    llp = logits64 - (np.log(np.exp(logits64 - lmax).sum()) + lmax)

    tab = np.zeros((D, CAUG), np.float32)
    tab[:, 0:C] = W
    tab[:, C:2 * C] = lp_tab.astype(np.float32)
    tab[:, 2 * C] = llp.astype(np.float32)

    in_maps = []
    for c in range(NC_COUNT):
        csl = slice(c * PER, (c + 1) * PER)
        in_maps.append({
            "pt": np.ascontiguousarray(pt_pack[:, :, csl]),
            "wth": wth, "wtl": wtl, "hb": hb, "ones2": ones2, "tab": tab,
        })
    return in_maps


def kernel(params, param_table, logits, _trace=False):
    global LAST_RESULTS
    from concourse.bass_utils import run_bass_kernel_spmd

    in_maps = _host_prep(params, param_table, logits)
    nc = _get_program()
    res = run_bass_kernel_spmd(nc, in_maps, core_ids=list(range(NC_COUNT)),
                               trace=_trace)
    LAST_RESULTS = res

    out2 = np.concatenate([r["out2"] for r in res.results], axis=0)
    bits = np.concatenate([r["bits_out"] for r in res.results], axis=0)

    params_quantized = np.ascontiguousarray(out2[:, 0:C]).reshape(B, N, C)
    log_pmf = np.ascontiguousarray(out2[:, C:2 * C]).reshape(B, N, C)
    param_bit = np.float32(bits.astype(np.float64).sum() / (-LN2))
    return log_pmf, params_quantized, param_bit
